# revision 1
# baseline (speedup 1.0000x reference)
"""GraphSAGE 2-layer GNN kernel for Trainium2 (8 NeuronCores, SPMD).

Strategy (dst-sharded graph parallel):
  - Nodes are permuted (degree-balanced round-robin) and partitioned into
    392 tiles of 128 nodes; each of the 8 cores owns 49 tiles (6272 padded
    node slots).
  - Edges are grouped by destination tile.  Each core processes the edges
    of its own dst tiles: it gathers source-node feature rows (bf16,
    256 B/row) from a replicated DRAM table with `dma_gather`, builds
    one-hot (dst-slot x node) selector tiles scaled by 1/deg on the vector
    engine, and aggregates with chunk matmuls accumulating in PSUM
    (aggT[d, n] += msgs.T @ onehot).
  - Layer output is computed transposed: hT = W_self.T @ xT + W_neigh.T @
    aggmeanT (+bias, relu) so no transposes are needed in the matmul chain.
  - Between layers, per-core hidden shards (converted node-major via DMA
    transpose) are AllGathered into a replicated DRAM table that layer 2
    gathers from.
  - int16 gather indices cannot address all 50176 padded rows, so the
    table is split in two overlapping index windows: A = rows [0, 32768),
    B = rows [17408, 50176); every edge uses whichever window contains
    its source row.

The final output is returned transposed per core ([64, 6272] fp32); the
host assembles/unpermutes to the full [50000, 64] result.
"""

import math

import numpy as np
import ml_dtypes

N = 50000
E = 800000
D_IN = 128
D_HID = 128
D_OUT = 64
CORES = 8
P = 128

NT = 49                    # node tiles per core
NPC = NT * P               # padded nodes per core (6272)
NTILES = CORES * NT        # 392 total tiles
NPAD = CORES * NPC         # 50176 padded rows total
HALF_A_END = 32768         # window A covers rows [0, 32768)
HALF_B_OFF = NPAD - 32768  # 17408; window B covers rows [17408, 50176)
GROUP = 7                  # tiles per gather call
NG = NT // GROUP           # 7 gather groups per layer

BF16 = ml_dtypes.bfloat16


def _prep(x, src, dst, W1_self, W1_neigh, b1, W2_self, W2_neigh, b2):
    """Host-side preprocessing: permutation, edge slotting, per-core arrays."""
    x = np.asarray(x, np.float32)
    src = np.asarray(src, np.int64)
    dst = np.asarray(dst, np.int64)

    deg = np.bincount(dst, minlength=N)
    invdeg = 1.0 / np.maximum(deg, 1).astype(np.float32)

    # Degree-balanced node -> (tile, slot) assignment: round-robin nodes in
    # descending-degree order over all 392 tiles. 392*128 == 50176 >= N.
    order = np.argsort(-deg, kind="stable")
    ranks = np.arange(N)
    tile_of = ranks % NTILES
    slot_of = ranks // NTILES
    g_of_node = np.empty(N, np.int64)
    g_of_node[order] = tile_of * P + slot_of  # tile-major global padded id
    node_of_g = np.full(NPAD, -1, np.int64)
    node_of_g[g_of_node] = np.arange(N)

    gsrc = g_of_node[src]
    gdst = g_of_node[dst]

    core_e = gdst // NPC
    lt_e = (gdst % NPC) // P
    dloc_e = gdst % P
    half_e = (gsrc >= HALF_A_END).astype(np.int64)
    lidx_e = gsrc - half_e * HALF_B_OFF
    assert lidx_e.max() < 32768 and lidx_e.min() >= 0

    # group edges by (core, tile, half)
    key = ((core_e * NT + lt_e) * 2 + half_e).astype(np.int64)
    edge_order = np.argsort(key, kind="stable")
    key_s = key[edge_order]
    counts = np.bincount(key_s, minlength=NTILES * 2)
    cnt_a = counts[0::2]
    cnt_b = counts[1::2]
    SA = int(math.ceil(cnt_a.max() / P) * P)  # A slots per tile
    SB = int(math.ceil(cnt_b.max() / P) * P)  # B slots per tile
    CA = SA // P
    CB = SB // P
    TS = NT * (SA + SB)      # slots per core
    TC = TS // P             # chunks per core

    starts = np.zeros(NTILES * 2, np.int64)
    starts[1:] = np.cumsum(counts)[:-1]
    within = np.arange(E) - starts[key_s]  # index within its group

    ks = key_s
    core_s = ks // (2 * NT)
    lt_s = (ks // 2) % NT
    half_s = ks % 2
    slot_s = np.where(half_s == 0, lt_s * SA + within,
                      NT * SA + lt_s * SB + within)

    idx_arr = np.zeros((CORES, TS), np.int16)
    # pad slots: dloc sentinel 300 never matches any column 0..127 (exact
    # in bf16), so padded gather rows contribute nothing to the one-hot MM
    dloc_arr = np.full((CORES, TS), 300.0, np.float32)
    invd_arr = np.zeros((CORES, TS), np.float32)
    flat = core_s * TS + slot_s
    idx_flat = idx_arr.reshape(-1)
    dloc_flat = dloc_arr.reshape(-1)
    invd_flat = invd_arr.reshape(-1)
    idx_flat[flat] = lidx_e[edge_order].astype(np.int16)
    dloc_flat[flat] = dloc_e[edge_order].astype(np.float32)
    invd_flat[flat] = invdeg[dst[edge_order]]

    # wrapped index layout: slot i -> [i % 16, i // 16], tiled to 128 rows
    idx_w = np.ascontiguousarray(
        np.tile(idx_arr.reshape(CORES, TS // 16, 16).transpose(0, 2, 1), (1, 8, 1))
    )
    # per-chunk scalar layout: slot i -> [i % 128, i // 128]
    dloc_w = np.ascontiguousarray(
        dloc_arr.reshape(CORES, TC, P).transpose(0, 2, 1)).astype(BF16)
    # per-node 1/deg, broadcast across partitions, per core shard [128, NPC]
    invdeg_pad = np.zeros(NPAD, np.float32)
    invdeg_pad[g_of_node] = invdeg
    invdb = [np.ascontiguousarray(
        np.tile(invdeg_pad[c * NPC:(c + 1) * NPC], (P, 1)))
        for c in range(CORES)]

    # feature tables / shards in padded-permuted order
    xpad = np.zeros((NPAD, D_IN), np.float32)
    xpad[g_of_node] = x
    x_bf = xpad.astype(BF16)
    xT_shards = [
        np.ascontiguousarray(xpad[c * NPC:(c + 1) * NPC].T) for c in range(CORES)
    ]

    CAmax = max(CA, CB)
    iota = np.tile(np.arange(P, dtype=np.float32), (P, CAmax)).astype(BF16)

    meta = dict(SA=SA, SB=SB, CA=CA, CB=CB, TS=TS, TC=TC,
                node_of_g=node_of_g, g_of_node=g_of_node)

    common = {
        "x_bf": x_bf,
        "iota": np.ascontiguousarray(iota),
        "W1s": np.ascontiguousarray(np.asarray(W1_self, np.float32)),
        "W1n": np.ascontiguousarray(np.asarray(W1_neigh, np.float32).astype(BF16)),
        "b1": np.ascontiguousarray(np.asarray(b1, np.float32).reshape(D_HID, 1)),
        "W2s": np.ascontiguousarray(np.asarray(W2_self, np.float32)),
        "W2n": np.ascontiguousarray(np.asarray(W2_neigh, np.float32).astype(BF16)),
        "b2": np.ascontiguousarray(np.asarray(b2, np.float32).reshape(D_OUT, 1)),
    }
    per_core = []
    for c in range(CORES):
        m = dict(common)
        m["xT"] = xT_shards[c]
        m["idx_w"] = idx_w[c]
        m["dloc"] = dloc_w[c]
        m["invdb"] = invdb[c]
        per_core.append(m)
    return per_core, meta


def _build(meta):
    """Build the SPMD Bass program (same NEFF for all 8 cores)."""
    import concourse.bacc as bacc
    import concourse.bass as bass
    import concourse.mybir as mybir
    import concourse.tile as tile

    CA, CB, TS, TC = meta["CA"], meta["CB"], meta["TS"], meta["TC"]
    SA, SB = meta["SA"], meta["SB"]
    f32 = mybir.dt.float32
    bf16 = mybir.dt.bfloat16
    i16 = mybir.dt.int16
    AF = mybir.ActivationFunctionType
    ALU = mybir.AluOpType

    nc = bacc.Bacc(None, target_bir_lowering=False, debug=False,
                   num_devices=CORES, num_swdge_queues=4)

    # I/O
    x_bf_t = nc.dram_tensor("x_bf", [NPAD, D_IN], bf16, kind="ExternalInput")
    xT_t = nc.dram_tensor("xT", [P, NPC], f32, kind="ExternalInput")
    idx_t = nc.dram_tensor("idx_w", [P, TS // 16], i16, kind="ExternalInput")
    CAmax = max(CA, CB)
    dloc_t = nc.dram_tensor("dloc", [P, TC], bf16, kind="ExternalInput")
    invdb_t = nc.dram_tensor("invdb", [P, NPC], f32, kind="ExternalInput")
    iota_t = nc.dram_tensor("iota", [P, CAmax * P], bf16, kind="ExternalInput")
    W1s_t = nc.dram_tensor("W1s", [D_IN, D_HID], f32, kind="ExternalInput")
    W1n_t = nc.dram_tensor("W1n", [D_IN, D_HID], bf16, kind="ExternalInput")
    b1_t = nc.dram_tensor("b1", [D_HID, 1], f32, kind="ExternalInput")
    W2s_t = nc.dram_tensor("W2s", [D_HID, D_OUT], f32, kind="ExternalInput")
    W2n_t = nc.dram_tensor("W2n", [D_HID, D_OUT], bf16, kind="ExternalInput")
    b2_t = nc.dram_tensor("b2", [D_OUT, 1], f32, kind="ExternalInput")
    out_t = nc.dram_tensor("outT", [D_OUT, NPC], f32, kind="ExternalOutput")

    h_shard_t = nc.dram_tensor("h_shard", [NPC, D_HID], bf16)
    h_table_t = nc.dram_tensor("h_table", [NPAD, D_HID], bf16,
                               addr_space="Shared")

    with tile.TileContext(nc) as tc:
        with (
            tc.tile_pool(name="const", bufs=1) as cpool,
            tc.tile_pool(name="msgsA", bufs=3) as poolA,
            tc.tile_pool(name="msgsB", bufs=3) as poolB,
            tc.tile_pool(name="oh", bufs=3) as pool_oh,
            tc.tile_pool(name="aggm", bufs=3) as pool_aggm,
            tc.tile_pool(name="small", bufs=3) as pool_small,
            tc.tile_pool(name="psA", bufs=2, space="PSUM") as psumA,
            tc.tile_pool(name="psH", bufs=2, space="PSUM") as psumH,
        ):
            # ---- persistent SBUF state -------------------------------------
            iota_sb = cpool.tile([P, CAmax, P], bf16)
            nc.sync.dma_start(iota_sb[:, :, :], iota_t[:].rearrange(
                "p (c n) -> p c n", n=P))
            idx_sb = cpool.tile([P, TS // 16], i16)
            nc.sync.dma_start(idx_sb[:], idx_t[:])
            dloc_sb = cpool.tile([P, TC], bf16)
            nc.sync.dma_start(dloc_sb[:], dloc_t[:])
            invdb_sb = cpool.tile([P, NPC], f32)
            nc.sync.dma_start(invdb_sb[:], invdb_t[:])
            xT_sb = cpool.tile([P, NPC], f32)
            nc.sync.dma_start(xT_sb[:], xT_t[:])
            hT_sb = cpool.tile([P, NPC], f32)
            W1s_sb = cpool.tile([D_IN, D_HID], f32)
            nc.sync.dma_start(W1s_sb[:], W1s_t[:])
            W1n_sb = cpool.tile([D_IN, D_HID], bf16)
            nc.sync.dma_start(W1n_sb[:], W1n_t[:])
            b1_sb = cpool.tile([D_HID, 1], f32)
            nc.sync.dma_start(b1_sb[:], b1_t[:])
            W2s_sb = cpool.tile([D_HID, D_OUT], f32)
            nc.sync.dma_start(W2s_sb[:], W2s_t[:])
            W2n_sb = cpool.tile([D_HID, D_OUT], bf16)
            nc.sync.dma_start(W2n_sb[:], W2n_t[:])
            b2_sb = cpool.tile([D_OUT, 1], f32)
            nc.sync.dma_start(b2_sb[:], b2_t[:])

            qrr = [0]

            def layer(li, table_t):
                for g in range(NG):
                    msA = poolA.tile([P, GROUP * CA, P], bf16, name=f"msA{li}{g}",
                                     tag="msA")
                    msB = poolB.tile([P, GROUP * CB, P], bf16, name=f"msB{li}{g}",
                                     tag="msB")
                    # gather calls are capped at 1024 indices: a single
                    # dma_gather with more overflows the SWDGE descriptor
                    # ring (ucode illegal_instruction around ~1100+ idxs).
                    def gcalls(ms, lt, t, S, C, region0, win):
                        s0 = region0 + t * S
                        done = 0
                        while done < S:
                            n = min(1024, S - done)
                            a = s0 + done
                            nc.gpsimd.dma_gather(
                                out_ap=ms[:, lt * C + done // P:
                                          lt * C + (done + n) // P, :],
                                in_ap=win,
                                idxs_ap=idx_sb[:, a // 16:(a + n) // 16],
                                num_idxs=n,
                                num_idxs_reg=n,
                                elem_size=D_IN,
                                queue_num=qrr[0] % 4,
                            )
                            qrr[0] += 1
                            done += n
                    for lt in range(GROUP):
                        t = g * GROUP + lt
                        gcalls(msA, lt, t, SA, CA, 0,
                               table_t[0:HALF_A_END, :])
                        gcalls(msB, lt, t, SB, CB, NT * SA,
                               table_t[HALF_B_OFF:NPAD, :])
                    for lt in range(GROUP):
                        t = g * GROUP + lt
                        agg = psumA.tile([P, P], f32, name=f"agg{li}{t}", tag="agg")
                        ohA = pool_oh.tile([P, CA, P], bf16, name=f"ohA{li}{t}",
                                           tag="ohA")
                        nc.vector.tensor_tensor(
                            out=ohA[:, :, :], in0=iota_sb[:, :CA, :],
                            in1=dloc_sb[:, t * CA:(t + 1) * CA].to_broadcast(
                                [P, CA, P]),
                            op=ALU.is_equal)
                        ohB = pool_oh.tile([P, CB, P], bf16, name=f"ohB{li}{t}",
                                           tag="ohB")
                        b0c = NT * CA + t * CB
                        nc.vector.tensor_tensor(
                            out=ohB[:, :, :], in0=iota_sb[:, :CB, :],
                            in1=dloc_sb[:, b0c:b0c + CB].to_broadcast(
                                [P, CB, P]),
                            op=ALU.is_equal)
                        nchunks = CA + CB
                        for c in range(nchunks):
                            if c < CA:
                                lhs = msA[:, lt * CA + c, :]
                                rhs = ohA[:, c, :]
                            else:
                                lhs = msB[:, lt * CB + (c - CA), :]
                                rhs = ohB[:, c - CA, :]
                            nc.tensor.matmul(
                                out=agg[:],
                                lhsT=lhs,
                                rhs=rhs,
                                start=(c == 0),
                                stop=(c == nchunks - 1),
                            )
                        aggm = pool_aggm.tile([P, P], bf16, name=f"am{li}{t}",
                                              tag="aggm")
                        ncol = slice(t * P, (t + 1) * P)
                        nc.vector.tensor_tensor(out=aggm[:], in0=agg[:],
                                                in1=invdb_sb[:, ncol],
                                                op=ALU.mult)
                        if li == 0:
                            hps = psumH.tile([P, P], f32, name=f"h{t}", tag="hps")
                            nc.tensor.matmul(out=hps[:], lhsT=W1n_sb[:],
                                             rhs=aggm[:], start=True, stop=False)
                            nc.tensor.matmul(out=hps[:], lhsT=W1s_sb[:],
                                             rhs=xT_sb[:, ncol],
                                             start=False, stop=True)
                            nc.scalar.activation(hT_sb[:, ncol], hps[:], AF.Relu,
                                                 bias=b1_sb[:, 0:1])
                            hbf = pool_small.tile([P, P], bf16, name=f"hb{t}",
                                                  tag="hbf")
                            nc.scalar.activation(hbf[:], hps[:], AF.Relu,
                                                 bias=b1_sb[:, 0:1])
                            hnode = pool_small.tile([P, P], bf16, name=f"hn{t}",
                                                    tag="hnode")
                            nc.sync.dma_start_transpose(hnode[:], hbf[:])
                            nc.sync.dma_start(
                                out=h_shard_t[t * P:(t + 1) * P, :], in_=hnode[:])
                        else:
                            ops = psumH.tile([D_OUT, P], f32, name=f"o{t}",
                                             tag="hps")
                            nc.tensor.matmul(out=ops[:], lhsT=W2n_sb[:],
                                             rhs=aggm[:], start=True, stop=False)
                            nc.tensor.matmul(out=ops[:], lhsT=W2s_sb[:],
                                             rhs=hT_sb[:, ncol],
                                             start=False, stop=True)
                            osb = pool_small.tile([D_OUT, P], f32, name=f"os{t}",
                                                  tag="osb")
                            nc.scalar.activation(osb[:], ops[:], AF.Identity,
                                                 bias=b2_sb[:, 0:1])
                            nc.sync.dma_start(out=out_t[:, ncol], in_=osb[:])

            layer(0, x_bf_t)
            nc.gpsimd.collective_compute(
                "AllGather",
                mybir.AluOpType.bypass,
                replica_groups=[list(range(CORES))],
                ins=[h_shard_t[:, :]],
                outs=[h_table_t[:, :]],
            )
            layer(1, h_table_t)

    nc.compile()
    return nc


_CACHE = {}


def kernel(x, src, dst, W1_self, W1_neigh, b1, W2_self, W2_neigh, b2,
           _want_perf=False):
    from concourse.bass_utils import run_bass_kernel_spmd

    per_core, meta = _prep(x, src, dst, W1_self, W1_neigh, b1,
                           W2_self, W2_neigh, b2)

    ck = (meta["SA"], meta["SB"])
    if ck not in _CACHE:
        _CACHE[ck] = _build(meta)
    nc = _CACHE[ck]

    res = run_bass_kernel_spmd(nc, per_core, core_ids=list(range(CORES)),
                               trace=_want_perf)

    node_of_g = meta["node_of_g"]
    outT = np.concatenate([r["outT"] for r in res.results], axis=1)  # [64, NPAD]
    out = np.empty((N, D_OUT), np.float32)
    valid = node_of_g >= 0
    out[node_of_g[valid]] = outT.T[valid]
    if _want_perf:
        return out, res
    return out

